# revision 8
# baseline (speedup 1.0000x reference)
"""Multi-head attention (B=4, N=2048, E=768, H=12, D=64) on 8 TRN2 NeuronCores.

Sharding: data-parallel on batch (4 batches x 2 cores each), tensor-parallel on
heads (6 heads per core).  Each core computes its heads' full NxN attention.
Partial output projections from the two cores of a batch are summed on the host.

Math simplifications (all exact):
  - softmax is shift invariant -> drop the +1.0 score bias and max-subtraction
    (scores are ~N(0,1); exp never overflows fp32)
  - K bias adds a per-query constant to every score row -> softmax invariant -> dropped
  - V bias shifts every attention output row by bv (softmax rows sum to 1)
    -> folded into the output bias on the host: b_eff = b_out + w_out @ bv
  - q scaling (1/8) folded into Wq and bq on the host

Device layout: scores are computed transposed (S^T = K Q^T, partition = key),
so P^T = exp(S^T) feeds the P@V matmul directly as the moving operand with
V as the stationary operand (O^T = (P V)^T accumulated over key blocks).
A ones-column appended to V (M=65) yields the softmax row-sums in the same
PE stream.  Matmuls run in fp32r (one-pass FP22).
"""

import sys

if "/opt/trn_rl_repo" not in sys.path:
    sys.path.insert(0, "/opt/trn_rl_repo")

import numpy as np

B, N, E = 4, 2048, 768
H, D = 12, 64
HPC = 6                     # heads per core
FQK = HPC * D               # 384 q (or k) features per core
NCORES = 8
SCALE = D ** -0.5

_CACHE = {}


def _build_bass():
    """Build the SPMD Bass program (same program on all 8 cores)."""
    if "nc" in _CACHE:
        return _CACHE["nc"]

    from contextlib import ExitStack

    import concourse.bass as bass
    import concourse.tile as tile
    from concourse import bacc, mybir

    f32 = mybir.dt.float32
    f32r = mybir.dt.float32r
    Identity = mybir.ActivationFunctionType.Identity
    Exp = mybir.ActivationFunctionType.Exp

    nc = bacc.Bacc(
        "TRN2",
        target_bir_lowering=False,
        debug=False,
        num_devices=NCORES,
    )

    xT = nc.dram_tensor("xT", (E, N), f32r, kind="ExternalInput").ap()        # x[b].T
    wqkT = nc.dram_tensor("wqkT", (E, 2 * FQK), f32r, kind="ExternalInput").ap()
    bq = nc.dram_tensor("bq", (FQK, 1), f32, kind="ExternalInput").ap()
    wvT = nc.dram_tensor("wvT", (E, FQK), f32r, kind="ExternalInput").ap()
    woT = nc.dram_tensor("woT", (FQK, E), f32r, kind="ExternalInput").ap()
    yp = nc.dram_tensor("yp", (N, E), f32, kind="ExternalOutput").ap()

    P = 128
    NCHUNK = 512            # token chunk for the projections
    IC = 1024               # query chunk in attention
    NPAIRS = HPC // 2       # head pairs (row-packed in the PE array)

    with ExitStack() as ctx:
        tc = ctx.enter_context(tile.TileContext(nc))

        # ---- persistent weight tiles -------------------------------------
        wpool = ctx.enter_context(tc.tile_pool(name="w", bufs=1))
        wqk_t = []
        wv_t = []
        for t in range(6):
            wqk = wpool.tile([P, 2 * FQK], f32r, tag=f"wqk{t}", name=f"wqk{t}")
            nc.sync.dma_start(wqk[:], wqkT[t * P:(t + 1) * P, :])
            wqk_t.append(wqk)
            wv = wpool.tile([P, FQK], f32r, tag=f"wv{t}", name=f"wv{t}")
            nc.sync.dma_start(wv[:], wvT[t * P:(t + 1) * P, :])
            wv_t.append(wv)
        wo_t = []
        for t in range(3):
            wo = wpool.tile([P, E], f32r, tag=f"wo{t}", name=f"wo{t}")
            nc.sync.dma_start(wo[:], woT[t * P:(t + 1) * P, :])
            wo_t.append(wo)
        bqt = wpool.tile([P, 3], f32, tag="bq", name="bqt")
        for fb in range(3):
            nc.sync.dma_start(bqt[:, fb:fb + 1], bq[fb * P:(fb + 1) * P, :])

        # ---- persistent activation tiles ---------------------------------
        qk_pool = ctx.enter_context(tc.tile_pool(name="qk", bufs=1))
        # f-blocks 0..2 = q features (heads 2fb, 2fb+1), 3..5 = k features
        qkT_t = [
            qk_pool.tile([P, N], f32r, tag=f"qk{fb}", name=f"qkT{fb}")
            for fb in range(6)
        ]
        v_pool = ctx.enter_context(tc.tile_pool(name="v", bufs=1))
        # V' per key-block: [128 keys, 6*65] = per head 64 V columns + a ones col
        v_t = [
            v_pool.tile([P, HPC * 65], f32r, tag=f"v{nb}", name=f"vv{nb}")
            for nb in range(N // P)
        ]

        # ---- PSUM pool ---------------------------------------------------
        # tags sA/sB: [128,1024] (2 banks each); tags oA/oB: [65,1024] (2 banks
        # each) -> 8 banks total.  Phase A/C matmul outputs reuse sA/sB slots.
        psum = ctx.enter_context(tc.tile_pool(name="ps", bufs=1, space="PSUM"))

        # ================= phase A: projections ===========================
        with tc.tile_pool(name="x", bufs=2) as xpool:
            for c4 in range(N // NCHUNK):
                n0 = c4 * NCHUNK
                xt = xpool.tile([P, 6 * NCHUNK], f32r, tag="x", name="xt")
                for t in range(6):
                    nc.sync.dma_start(
                        xt[:, t * NCHUNK:(t + 1) * NCHUNK],
                        xT[t * P:(t + 1) * P, n0:n0 + NCHUNK],
                    )
                # q/k features, transposed: qkT[f, n]
                for fb in range(6):
                    ps = psum.tile(
                        [P, NCHUNK], f32, tag=("sA" if fb % 2 == 0 else "sB"),
                        name="ps_qk",
                    )
                    for et in range(6):
                        nc.tensor.matmul(
                            ps[:],
                            lhsT=wqk_t[et][:, fb * P:(fb + 1) * P],
                            rhs=xt[:, et * NCHUNK:(et + 1) * NCHUNK],
                            start=(et == 0),
                            stop=(et == 5),
                        )
                    dst = qkT_t[fb][:, n0:n0 + NCHUNK]
                    if fb < 3:
                        nc.scalar.activation(
                            dst, ps[:], Identity, bias=bqt[:, fb:fb + 1]
                        )
                    else:
                        nc.scalar.copy(dst, ps[:])
                # V natural: V[n, d] for this chunk's 4 key-blocks
                for nb2 in range(NCHUNK // P):
                    nb = c4 * (NCHUNK // P) + nb2
                    psv = psum.tile(
                        [P, FQK], f32, tag=("sA" if nb2 % 2 == 0 else "sB"),
                        name="ps_v",
                    )
                    for et in range(6):
                        nc.tensor.matmul(
                            psv[:],
                            lhsT=xt[:, et * NCHUNK + nb2 * P:
                                      et * NCHUNK + (nb2 + 1) * P],
                            rhs=wv_t[et][:],
                            start=(et == 0),
                            stop=(et == 5),
                        )
                    # ones-fill first, then copy V into the 64-wide slices;
                    # column 64 of each 65-group stays 1.0 (rowsum column)
                    nc.gpsimd.memset(v_t[nb].bitcast(f32)[:], 1.0)
                    v3 = v_t[nb].rearrange("p (h c) -> p h c", c=65)
                    nc.vector.tensor_copy(
                        v3[:, :, 0:64],
                        psv.rearrange("p (h c) -> p h c", c=64),
                    )

        # ================= phase B: attention =============================
        pt_pool = ctx.enter_context(tc.tile_pool(name="pt", bufs=3))
        oT_pool = ctx.enter_context(tc.tile_pool(name="oT", bufs=1))
        # pair p partitions 0:64 = head 2p, 64:128 = head 2p+1
        oT_t = [
            oT_pool.tile([P, N], f32r, tag=f"oT{p}", name=f"oT{p}")
            for p in range(NPAIRS)
        ]
        nrm_pool = ctx.enter_context(tc.tile_pool(name="nrm", bufs=2))

        for p in range(NPAIRS):
            qT = qkT_t[p]
            kT = qkT_t[3 + p]
            for ic in range(N // IC):
                i0 = ic * IC
                oA = psum.tile([65, IC], f32, tag="oA", name="oA")
                oB = psum.tile([65, IC], f32, tag="oB", name="oB")
                for jb in range(N // P):
                    j0 = jb * P
                    sA = psum.tile([P, IC], f32, tag="sA", name="sA")
                    sB = psum.tile([P, IC], f32, tag="sB", name="sB")
                    ptA = pt_pool.tile([P, IC], f32r, tag="ptA", name="ptA")
                    ptB = pt_pool.tile([P, IC], f32r, tag="ptB", name="ptB")
                    for u in range(IC // 512):
                        nc.tensor.matmul(
                            sA[:, u * 512:(u + 1) * 512],
                            lhsT=kT[0:64, j0:j0 + P],
                            rhs=qT[0:64, i0 + u * 512:i0 + (u + 1) * 512],
                            start=True,
                            stop=True,
                        )
                    nc.scalar.activation(ptA[:], sA[:], Exp)
                    for u in range(IC // 512):
                        nc.tensor.matmul(
                            sB[:, u * 512:(u + 1) * 512],
                            lhsT=kT[64:128, j0:j0 + P],
                            rhs=qT[64:128, i0 + u * 512:i0 + (u + 1) * 512],
                            start=True,
                            stop=True,
                        )
                    nc.scalar.activation(ptB[:], sB[:], Exp)
                    for u in range(IC // 512):
                        nc.tensor.matmul(
                            oA[:, u * 512:(u + 1) * 512],
                            lhsT=v_t[jb][:, (2 * p) * 65:(2 * p) * 65 + 65],
                            rhs=ptA[:, u * 512:(u + 1) * 512],
                            start=(jb == 0),
                            stop=(jb == N // P - 1),
                        )
                        nc.tensor.matmul(
                            oB[:, u * 512:(u + 1) * 512],
                            lhsT=v_t[jb][:, (2 * p + 1) * 65:(2 * p + 1) * 65 + 65],
                            rhs=ptB[:, u * 512:(u + 1) * 512],
                            start=(jb == 0),
                            stop=(jb == N // P - 1),
                        )
                # softmax normalization: O^T[d, i] /= rowsum[i]
                for half, o_ps in ((0, oA), (1, oB)):
                    rs = nrm_pool.tile([1, IC], f32, tag="rs", name="rs")
                    nc.vector.tensor_copy(rs[:], o_ps[64:65, :])
                    rs128 = nrm_pool.tile([P, IC // P], f32, tag="rs128",
                                          name="rs128")
                    nc.sync.dma_start(rs128[:], rs[:])
                    rcp = nrm_pool.tile([P, IC // P], f32, tag="rcp", name="rcp")
                    nc.vector.reciprocal(rcp[:], rs128[:])
                    rcpf = nrm_pool.tile([1, IC], f32, tag="rcpf", name="rcpf")
                    nc.sync.dma_start(rcpf[:], rcp[:])
                    rb = nrm_pool.tile([64, IC], f32, tag="rb", name="rb")
                    nc.gpsimd.partition_broadcast(rb[:], rcpf[:])
                    nc.vector.tensor_mul(
                        oT_t[p][half * 64:(half + 1) * 64, i0:i0 + IC],
                        o_ps[0:64, :],
                        rb[:],
                    )

        # ================= phase C: output projection =====================
        y_pool = ctx.enter_context(tc.tile_pool(name="y", bufs=2))
        for nb in range(N // P):
            n0 = nb * P
            yt = y_pool.tile([P, E], f32, tag="y", name="yt")
            for half in range(2):
                f0 = half * 384
                psy = psum.tile(
                    [P, 384], f32, tag=("sA" if half == 0 else "sB"), name="psy"
                )
                for dt3 in range(3):
                    nc.tensor.matmul(
                        psy[:],
                        lhsT=oT_t[dt3][:, n0:n0 + P],
                        rhs=wo_t[dt3][:, f0:f0 + 384],
                        start=(dt3 == 0),
                        stop=(dt3 == 2),
                    )
                nc.vector.tensor_copy(yt[:, f0:f0 + 384], psy[:])
            nc.sync.dma_start(yp[n0:n0 + P, :], yt[:])

    nc.compile()
    _CACHE["nc"] = nc
    return nc


def _shard_inputs(x_q, w_qkv, b_qkv, w_out):
    """Build the 8 per-core input maps (numpy, host side)."""
    in_maps = []
    for c in range(NCORES):
        b = c // 2
        h0 = (c % 2) * HPC
        qs = slice(h0 * D, h0 * D + FQK)
        ks = slice(E + h0 * D, E + h0 * D + FQK)
        vs = slice(2 * E + h0 * D, 2 * E + h0 * D + FQK)
        wq = w_qkv[qs] * SCALE                       # (384, 768)
        wk = w_qkv[ks]
        wv = w_qkv[vs]
        in_maps.append({
            "xT": np.ascontiguousarray(x_q[b].T),                    # (768, 2048)
            "wqkT": np.ascontiguousarray(
                np.concatenate([wq, wk], axis=0).T),                 # (768, 768)
            "bq": np.ascontiguousarray(
                (b_qkv[qs] * SCALE).reshape(FQK, 1)),                # (384, 1)
            "wvT": np.ascontiguousarray(wv.T),                       # (768, 384)
            "woT": np.ascontiguousarray(w_out[:, h0 * D:h0 * D + FQK].T),
        })
    return in_maps


def kernel(x_q, w_qkv, b_qkv, w_out, b_out, _trace=False, _tmpdir=None):
    x_q = np.asarray(x_q, dtype=np.float32)
    w_qkv = np.asarray(w_qkv, dtype=np.float32)
    b_qkv = np.asarray(b_qkv, dtype=np.float32)
    w_out = np.asarray(w_out, dtype=np.float32)
    b_out = np.asarray(b_out, dtype=np.float32)

    from concourse.bass_utils import run_bass_kernel_spmd

    nc = _build_bass()
    in_maps = _shard_inputs(x_q, w_qkv, b_qkv, w_out)
    res = run_bass_kernel_spmd(
        nc, in_maps, core_ids=list(range(NCORES)), trace=_trace, tmpdir=_tmpdir
    )
    _CACHE["last_result"] = res

    # host unshard: sum the two head-shards of each batch, add the folded bias
    bv = b_qkv[2 * E:]                       # v bias, folded through w_out
    b_eff = b_out + w_out @ bv               # (768,)
    y = np.empty((B, N, E), dtype=np.float32)
    for b in range(B):
        y[b] = res.results[2 * b]["yp"] + res.results[2 * b + 1]["yp"] + b_eff
    return y


# revision 10
# speedup vs baseline: 1.2280x; 1.2280x over previous
"""Multi-head attention (B=4, N=2048, E=768, H=12, D=64) on 8 TRN2 NeuronCores.

Sharding: data-parallel on batch (4 batches x 2 cores each), tensor-parallel on
heads (6 heads per core).  Each core computes its heads' full NxN attention.
Partial output projections from the two cores of a batch are summed on the host.

Math simplifications (all exact):
  - softmax is shift invariant -> drop the +1.0 score bias and max-subtraction
    (scores are ~N(0,1); exp never overflows fp32)
  - K bias adds a per-query constant to every score row -> softmax invariant -> dropped
  - V bias shifts every attention output row by bv (softmax rows sum to 1)
    -> folded into the output bias on the host: b_eff = b_out + w_out @ bv
  - q scaling (1/8) folded into Wq and bq on the host

Device layout: scores are computed transposed (S^T = K Q^T, partition = key),
so P^T = exp(S^T) feeds the P@V matmul directly as the moving operand with
V as the stationary operand (O^T = (P V)^T accumulated over key blocks).
A ones-column appended to V (M=65) yields the softmax row-sums in the same
PE stream.  Matmul operands are bf16 (fp32 PSUM accumulation) -- the PE
streams 2 bytes/cycle/partition, so 4-byte fp32r runs at half rate.  Set
PRECISION = "f32r" for full fp32r (~2x slower, ~10x lower error).
"""

import sys

if "/opt/trn_rl_repo" not in sys.path:
    sys.path.insert(0, "/opt/trn_rl_repo")

import numpy as np

B, N, E = 4, 2048, 768
H, D = 12, 64
HPC = 6                     # heads per core
FQK = HPC * D               # 384 q (or k) features per core
NCORES = 8
SCALE = D ** -0.5
PRECISION = "bf16"          # "bf16" | "f32r" (matmul operand dtype)

_CACHE = {}


def _build_bass():
    """Build the SPMD Bass program (same program on all 8 cores)."""
    if "nc" in _CACHE:
        return _CACHE["nc"]

    from contextlib import ExitStack

    import concourse.bass as bass
    import concourse.tile as tile
    from concourse import bacc, mybir

    f32 = mybir.dt.float32
    fmm = mybir.dt.bfloat16 if PRECISION == "bf16" else mybir.dt.float32r
    Identity = mybir.ActivationFunctionType.Identity
    Exp = mybir.ActivationFunctionType.Exp

    nc = bacc.Bacc(
        "TRN2",
        target_bir_lowering=False,
        debug=False,
        num_devices=NCORES,
    )

    xT = nc.dram_tensor("xT", (E, N), fmm, kind="ExternalInput").ap()        # x[b].T
    wqkT = nc.dram_tensor("wqkT", (E, 2 * FQK), fmm, kind="ExternalInput").ap()
    bq = nc.dram_tensor("bq", (FQK, 1), f32, kind="ExternalInput").ap()
    wvT = nc.dram_tensor("wvT", (E, FQK), fmm, kind="ExternalInput").ap()
    woT = nc.dram_tensor("woT", (FQK, E), fmm, kind="ExternalInput").ap()
    yp = nc.dram_tensor("yp", (N, E), f32, kind="ExternalOutput").ap()

    P = 128
    NCHUNK = 512            # token chunk for the projections
    IC = 1024               # query chunk in attention
    NPAIRS = HPC // 2       # head pairs (row-packed in the PE array)

    with ExitStack() as ctx:
        tc = ctx.enter_context(tile.TileContext(nc))

        # ---- persistent weight tiles -------------------------------------
        wpool = ctx.enter_context(tc.tile_pool(name="w", bufs=1))
        wqk_t = []
        wv_t = []
        for t in range(6):
            wqk = wpool.tile([P, 2 * FQK], fmm, tag=f"wqk{t}", name=f"wqk{t}")
            nc.sync.dma_start(wqk[:], wqkT[t * P:(t + 1) * P, :])
            wqk_t.append(wqk)
            wv = wpool.tile([P, FQK], fmm, tag=f"wv{t}", name=f"wv{t}")
            nc.sync.dma_start(wv[:], wvT[t * P:(t + 1) * P, :])
            wv_t.append(wv)
        wo_t = []
        for t in range(3):
            wo = wpool.tile([P, E], fmm, tag=f"wo{t}", name=f"wo{t}")
            nc.sync.dma_start(wo[:], woT[t * P:(t + 1) * P, :])
            wo_t.append(wo)
        bqt = wpool.tile([P, 3], f32, tag="bq", name="bqt")
        for fb in range(3):
            nc.sync.dma_start(bqt[:, fb:fb + 1], bq[fb * P:(fb + 1) * P, :])

        # ---- persistent activation tiles ---------------------------------
        qk_pool = ctx.enter_context(tc.tile_pool(name="qk", bufs=1))
        # f-blocks 0..2 = q features (heads 2fb, 2fb+1), 3..5 = k features
        qkT_t = [
            qk_pool.tile([P, N], fmm, tag=f"qk{fb}", name=f"qkT{fb}")
            for fb in range(6)
        ]
        v_pool = ctx.enter_context(tc.tile_pool(name="v", bufs=1))
        # V' per key-block: [128 keys, 6*65] = per head 64 V columns + a ones col
        v_t = [
            v_pool.tile([P, HPC * 65], fmm, tag=f"v{nb}", name=f"vv{nb}")
            for nb in range(N // P)
        ]

        # ---- PSUM pool ---------------------------------------------------
        # tags sA/sB: [128,1024] (2 banks each); tags oA/oB: [65,1024] (2 banks
        # each) -> 8 banks total.  Phase A/C matmul outputs reuse sA/sB slots.
        psum = ctx.enter_context(tc.tile_pool(name="ps", bufs=1, space="PSUM"))

        # ================= phase A: projections ===========================
        with tc.tile_pool(name="x", bufs=2) as xpool:
            for c4 in range(N // NCHUNK):
                n0 = c4 * NCHUNK
                xt = xpool.tile([P, 6 * NCHUNK], fmm, tag="x", name="xt")
                for t in range(6):
                    nc.sync.dma_start(
                        xt[:, t * NCHUNK:(t + 1) * NCHUNK],
                        xT[t * P:(t + 1) * P, n0:n0 + NCHUNK],
                    )
                # q/k features, transposed: qkT[f, n]
                for fb in range(6):
                    ps = psum.tile(
                        [P, NCHUNK], f32, tag=("sA" if fb % 2 == 0 else "sB"),
                        name="ps_qk",
                    )
                    for et in range(6):
                        nc.tensor.matmul(
                            ps[:],
                            lhsT=wqk_t[et][:, fb * P:(fb + 1) * P],
                            rhs=xt[:, et * NCHUNK:(et + 1) * NCHUNK],
                            start=(et == 0),
                            stop=(et == 5),
                        )
                    dst = qkT_t[fb][:, n0:n0 + NCHUNK]
                    if fb < 3:
                        nc.vector.tensor_scalar_add(dst, ps[:], bqt[:, fb:fb + 1])
                    else:
                        nc.vector.tensor_copy(dst, ps[:])
                # V natural: V[n, d] for this chunk's 4 key-blocks
                for nb2 in range(NCHUNK // P):
                    nb = c4 * (NCHUNK // P) + nb2
                    psv = psum.tile(
                        [P, FQK], f32, tag=("sA" if nb2 % 2 == 0 else "sB"),
                        name="ps_v",
                    )
                    for et in range(6):
                        nc.tensor.matmul(
                            psv[:],
                            lhsT=xt[:, et * NCHUNK + nb2 * P:
                                      et * NCHUNK + (nb2 + 1) * P],
                            rhs=wv_t[et][:],
                            start=(et == 0),
                            stop=(et == 5),
                        )
                    # ones-fill first, then copy V into the 64-wide slices;
                    # column 64 of each 65-group stays 1.0 (rowsum column)
                    if PRECISION == "bf16":
                        nc.gpsimd.memset(v_t[nb][:], 1.0)
                    else:
                        nc.gpsimd.memset(v_t[nb].bitcast(f32)[:], 1.0)
                    v3 = v_t[nb].rearrange("p (h c) -> p h c", c=65)
                    nc.vector.tensor_copy(
                        v3[:, :, 0:64],
                        psv.rearrange("p (h c) -> p h c", c=64),
                    )

        # ================= phase B: attention =============================
        pt_pool = ctx.enter_context(tc.tile_pool(name="pt", bufs=3))
        oT_pool = ctx.enter_context(tc.tile_pool(name="oT", bufs=1))
        # pair p partitions 0:64 = head 2p, 64:128 = head 2p+1
        oT_t = [
            oT_pool.tile([P, N], fmm, tag=f"oT{p}", name=f"oT{p}")
            for p in range(NPAIRS)
        ]
        nrm_pool = ctx.enter_context(tc.tile_pool(name="nrm", bufs=2))

        for p in range(NPAIRS):
            qT = qkT_t[p]
            kT = qkT_t[3 + p]
            for ic in range(N // IC):
                i0 = ic * IC
                oA = psum.tile([65, IC], f32, tag="oA", name="oA")
                oB = psum.tile([65, IC], f32, tag="oB", name="oB")
                for jb in range(N // P):
                    j0 = jb * P
                    sA = psum.tile([P, IC], f32, tag="sA", name="sA")
                    sB = psum.tile([P, IC], f32, tag="sB", name="sB")
                    ptA = pt_pool.tile([P, IC], fmm, tag="ptA", name="ptA")
                    ptB = pt_pool.tile([P, IC], fmm, tag="ptB", name="ptB")
                    for u in range(IC // 512):
                        nc.tensor.matmul(
                            sA[:, u * 512:(u + 1) * 512],
                            lhsT=kT[0:64, j0:j0 + P],
                            rhs=qT[0:64, i0 + u * 512:i0 + (u + 1) * 512],
                            start=True,
                            stop=True,
                        )
                    nc.scalar.activation(ptA[:], sA[:], Exp)
                    for u in range(IC // 512):
                        nc.tensor.matmul(
                            sB[:, u * 512:(u + 1) * 512],
                            lhsT=kT[64:128, j0:j0 + P],
                            rhs=qT[64:128, i0 + u * 512:i0 + (u + 1) * 512],
                            start=True,
                            stop=True,
                        )
                    nc.scalar.activation(ptB[:], sB[:], Exp)
                    for u in range(IC // 512):
                        nc.tensor.matmul(
                            oA[:, u * 512:(u + 1) * 512],
                            lhsT=v_t[jb][:, (2 * p) * 65:(2 * p) * 65 + 65],
                            rhs=ptA[:, u * 512:(u + 1) * 512],
                            start=(jb == 0),
                            stop=(jb == N // P - 1),
                        )
                        nc.tensor.matmul(
                            oB[:, u * 512:(u + 1) * 512],
                            lhsT=v_t[jb][:, (2 * p + 1) * 65:(2 * p + 1) * 65 + 65],
                            rhs=ptB[:, u * 512:(u + 1) * 512],
                            start=(jb == 0),
                            stop=(jb == N // P - 1),
                        )
                # softmax normalization: O^T[d, i] /= rowsum[i]
                for half, o_ps in ((0, oA), (1, oB)):
                    rs = nrm_pool.tile([1, IC], f32, tag="rs", name="rs")
                    nc.vector.tensor_copy(rs[:], o_ps[64:65, :])
                    rs128 = nrm_pool.tile([P, IC // P], f32, tag="rs128",
                                          name="rs128")
                    nc.sync.dma_start(rs128[:], rs[:])
                    rcp = nrm_pool.tile([P, IC // P], f32, tag="rcp", name="rcp")
                    nc.vector.reciprocal(rcp[:], rs128[:])
                    rcpf = nrm_pool.tile([1, IC], f32, tag="rcpf", name="rcpf")
                    nc.sync.dma_start(rcpf[:], rcp[:])
                    rb = nrm_pool.tile([64, IC], f32, tag="rb", name="rb")
                    nc.gpsimd.partition_broadcast(rb[:], rcpf[:])
                    nc.vector.tensor_mul(
                        oT_t[p][half * 64:(half + 1) * 64, i0:i0 + IC],
                        o_ps[0:64, :],
                        rb[:],
                    )

        # ================= phase C: output projection =====================
        y_pool = ctx.enter_context(tc.tile_pool(name="y", bufs=2))
        for nb in range(N // P):
            n0 = nb * P
            yt = y_pool.tile([P, E], f32, tag="y", name="yt")
            for half in range(2):
                f0 = half * 384
                psy = psum.tile(
                    [P, 384], f32, tag=("sA" if half == 0 else "sB"), name="psy"
                )
                for dt3 in range(3):
                    nc.tensor.matmul(
                        psy[:],
                        lhsT=oT_t[dt3][:, n0:n0 + P],
                        rhs=wo_t[dt3][:, f0:f0 + 384],
                        start=(dt3 == 0),
                        stop=(dt3 == 2),
                    )
                nc.vector.tensor_copy(yt[:, f0:f0 + 384], psy[:])
            nc.sync.dma_start(yp[n0:n0 + P, :], yt[:])

    nc.compile()
    _CACHE["nc"] = nc
    return nc


def _shard_inputs(x_q, w_qkv, b_qkv, w_out):
    """Build the 8 per-core input maps (numpy, host side)."""
    if PRECISION == "bf16":
        import ml_dtypes

        mm_np = ml_dtypes.bfloat16
    else:
        mm_np = np.float32

    def cmm(a):
        return np.ascontiguousarray(a.astype(mm_np))

    in_maps = []
    for c in range(NCORES):
        b = c // 2
        h0 = (c % 2) * HPC
        qs = slice(h0 * D, h0 * D + FQK)
        ks = slice(E + h0 * D, E + h0 * D + FQK)
        vs = slice(2 * E + h0 * D, 2 * E + h0 * D + FQK)
        wq = w_qkv[qs] * SCALE                       # (384, 768)
        wk = w_qkv[ks]
        wv = w_qkv[vs]
        in_maps.append({
            "xT": cmm(x_q[b].T),                                     # (768, 2048)
            "wqkT": cmm(np.concatenate([wq, wk], axis=0).T),         # (768, 768)
            "bq": np.ascontiguousarray(
                (b_qkv[qs] * SCALE).reshape(FQK, 1)),                # (384, 1)
            "wvT": cmm(wv.T),                                        # (768, 384)
            "woT": cmm(w_out[:, h0 * D:h0 * D + FQK].T),
        })
    return in_maps


def kernel(x_q, w_qkv, b_qkv, w_out, b_out, _trace=False, _tmpdir=None):
    x_q = np.asarray(x_q, dtype=np.float32)
    w_qkv = np.asarray(w_qkv, dtype=np.float32)
    b_qkv = np.asarray(b_qkv, dtype=np.float32)
    w_out = np.asarray(w_out, dtype=np.float32)
    b_out = np.asarray(b_out, dtype=np.float32)

    from concourse.bass_utils import run_bass_kernel_spmd

    nc = _build_bass()
    in_maps = _shard_inputs(x_q, w_qkv, b_qkv, w_out)
    res = run_bass_kernel_spmd(
        nc, in_maps, core_ids=list(range(NCORES)), trace=_trace, tmpdir=_tmpdir
    )
    _CACHE["last_result"] = res

    # host unshard: sum the two head-shards of each batch, add the folded bias
    bv = b_qkv[2 * E:]                       # v bias, folded through w_out
    b_eff = b_out + w_out @ bv               # (768,)
    y = np.empty((B, N, E), dtype=np.float32)
    for b in range(B):
        y[b] = res.results[2 * b]["yp"] + res.results[2 * b + 1]["yp"] + b_eff
    return y


# revision 11
# speedup vs baseline: 1.2354x; 1.0060x over previous
"""Multi-head attention (B=4, N=2048, E=768, H=12, D=64) on 8 TRN2 NeuronCores.

Sharding: data-parallel on batch (4 batches x 2 cores each), tensor-parallel on
heads (6 heads per core).  Each core computes its heads' full NxN attention.
Partial output projections from the two cores of a batch are summed on the host.

Math simplifications (all exact):
  - softmax is shift invariant -> drop the +1.0 score bias and max-subtraction
    (scores are ~N(0,1); exp never overflows fp32)
  - K bias adds a per-query constant to every score row -> softmax invariant -> dropped
  - V bias shifts every attention output row by bv (softmax rows sum to 1)
    -> folded into the output bias on the host: b_eff = b_out + w_out @ bv
  - q scaling (1/8) folded into Wq and bq on the host

Device layout: scores are computed transposed (S^T = K Q^T, partition = key),
so P^T = exp(S^T) feeds the P@V matmul directly as the moving operand with
V as the stationary operand (O^T = (P V)^T accumulated over key blocks).
A ones-column appended to V (M=65) yields the softmax row-sums in the same
PE stream.  Matmul operands are bf16 (fp32 PSUM accumulation) -- the PE
streams 2 bytes/cycle/partition, so 4-byte fp32r runs at half rate.  Set
PRECISION = "f32r" for full fp32r (~2x slower, ~10x lower error).
"""

import sys

if "/opt/trn_rl_repo" not in sys.path:
    sys.path.insert(0, "/opt/trn_rl_repo")

import numpy as np

B, N, E = 4, 2048, 768
H, D = 12, 64
HPC = 6                     # heads per core
FQK = HPC * D               # 384 q (or k) features per core
NCORES = 8
SCALE = D ** -0.5
PRECISION = "bf16"          # "bf16" | "f32r" (matmul operand dtype)

_CACHE = {}


def _build_bass():
    """Build the SPMD Bass program (same program on all 8 cores)."""
    if "nc" in _CACHE:
        return _CACHE["nc"]

    from contextlib import ExitStack

    import concourse.bass as bass
    import concourse.tile as tile
    from concourse import bacc, mybir

    f32 = mybir.dt.float32
    fmm = mybir.dt.bfloat16 if PRECISION == "bf16" else mybir.dt.float32r
    Identity = mybir.ActivationFunctionType.Identity
    Exp = mybir.ActivationFunctionType.Exp

    nc = bacc.Bacc(
        "TRN2",
        target_bir_lowering=False,
        debug=False,
        num_devices=NCORES,
    )

    xT = nc.dram_tensor("xT", (E, N), fmm, kind="ExternalInput").ap()        # x[b].T
    wqkT = nc.dram_tensor("wqkT", (E, 2 * FQK), fmm, kind="ExternalInput").ap()
    bq = nc.dram_tensor("bq", (FQK, 1), f32, kind="ExternalInput").ap()
    wvT = nc.dram_tensor("wvT", (E, FQK), fmm, kind="ExternalInput").ap()
    woT = nc.dram_tensor("woT", (FQK, E), fmm, kind="ExternalInput").ap()
    yp = nc.dram_tensor("yp", (N, E), f32, kind="ExternalOutput").ap()

    P = 128
    NCHUNK = 512            # token chunk for the projections
    IC = 1024               # query chunk in attention
    NPAIRS = HPC // 2       # head pairs (row-packed in the PE array)

    with ExitStack() as ctx:
        tc = ctx.enter_context(tile.TileContext(nc))

        # ---- persistent weight tiles -------------------------------------
        wpool = ctx.enter_context(tc.tile_pool(name="w", bufs=1))
        wqk_t = []
        wv_t = []
        for t in range(6):
            wqk = wpool.tile([P, 2 * FQK], fmm, tag=f"wqk{t}", name=f"wqk{t}")
            nc.sync.dma_start(wqk[:], wqkT[t * P:(t + 1) * P, :])
            wqk_t.append(wqk)
            wv = wpool.tile([P, FQK], fmm, tag=f"wv{t}", name=f"wv{t}")
            nc.sync.dma_start(wv[:], wvT[t * P:(t + 1) * P, :])
            wv_t.append(wv)
        bqt = wpool.tile([P, 3], f32, tag="bq", name="bqt")
        for fb in range(3):
            nc.sync.dma_start(bqt[:, fb:fb + 1], bq[fb * P:(fb + 1) * P, :])

        # ---- persistent activation tiles ---------------------------------
        qk_pool = ctx.enter_context(tc.tile_pool(name="qk", bufs=1))
        # f-blocks 0..2 = q features (heads 2fb, 2fb+1), 3..5 = k features
        qkT_t = [
            qk_pool.tile([P, N], fmm, tag=f"qk{fb}", name=f"qkT{fb}")
            for fb in range(6)
        ]
        v_pool = ctx.enter_context(tc.tile_pool(name="v", bufs=1))
        # V' per key-block: [128 keys, 6*65] = per head 64 V columns + a ones col
        v_t = [
            v_pool.tile([P, HPC * 65], fmm, tag=f"v{nb}", name=f"vv{nb}")
            for nb in range(N // P)
        ]

        # ---- PSUM pool ---------------------------------------------------
        # tags sA/sB: [128,1024] (2 banks each); tags oA/oB: [65,1024] (2 banks
        # each) -> 8 banks total.  Phase A/C matmul outputs reuse sA/sB slots.
        psum = ctx.enter_context(tc.tile_pool(name="ps", bufs=1, space="PSUM"))

        # ================= phase A: projections ===========================
        with tc.tile_pool(name="x", bufs=2) as xpool:
            for c4 in range(N // NCHUNK):
                n0 = c4 * NCHUNK
                xt = xpool.tile([P, 6 * NCHUNK], fmm, tag="x", name="xt")
                for t in range(6):
                    nc.sync.dma_start(
                        xt[:, t * NCHUNK:(t + 1) * NCHUNK],
                        xT[t * P:(t + 1) * P, n0:n0 + NCHUNK],
                    )
                # q/k features, transposed: qkT[f, n]
                for fb in range(6):
                    ps = psum.tile(
                        [P, NCHUNK], f32, tag=("sA" if fb % 2 == 0 else "sB"),
                        name="ps_qk",
                    )
                    for et in range(6):
                        nc.tensor.matmul(
                            ps[:],
                            lhsT=wqk_t[et][:, fb * P:(fb + 1) * P],
                            rhs=xt[:, et * NCHUNK:(et + 1) * NCHUNK],
                            start=(et == 0),
                            stop=(et == 5),
                        )
                    dst = qkT_t[fb][:, n0:n0 + NCHUNK]
                    if fb < 3:
                        nc.vector.tensor_scalar_add(dst, ps[:], bqt[:, fb:fb + 1])
                    else:
                        nc.vector.tensor_copy(dst, ps[:])
                # V natural: V[n, d] for this chunk's 4 key-blocks
                for nb2 in range(NCHUNK // P):
                    nb = c4 * (NCHUNK // P) + nb2
                    psv = psum.tile(
                        [P, FQK], f32, tag=("sA" if nb2 % 2 == 0 else "sB"),
                        name="ps_v",
                    )
                    for et in range(6):
                        nc.tensor.matmul(
                            psv[:],
                            lhsT=xt[:, et * NCHUNK + nb2 * P:
                                      et * NCHUNK + (nb2 + 1) * P],
                            rhs=wv_t[et][:],
                            start=(et == 0),
                            stop=(et == 5),
                        )
                    # ones-fill first, then copy V into the 64-wide slices;
                    # column 64 of each 65-group stays 1.0 (rowsum column)
                    if PRECISION == "bf16":
                        nc.gpsimd.memset(v_t[nb][:], 1.0)
                    else:
                        nc.gpsimd.memset(v_t[nb].bitcast(f32)[:], 1.0)
                    v3 = v_t[nb].rearrange("p (h c) -> p h c", c=65)
                    nc.vector.tensor_copy(
                        v3[:, :, 0:64],
                        psv.rearrange("p (h c) -> p h c", c=64),
                    )

        # output-projection weights load after the x/qkv weights (not needed
        # until the first out-projection)
        wo_t = []
        for t in range(3):
            wo = wpool.tile([P, E], fmm, tag=f"wo{t}", name=f"wo{t}")
            nc.sync.dma_start(wo[:], woT[t * P:(t + 1) * P, :])
            wo_t.append(wo)

        # ================= phase B: attention =============================
        pt_pool = ctx.enter_context(tc.tile_pool(name="pt", bufs=8))
        oT_pool = ctx.enter_context(tc.tile_pool(name="oT", bufs=1))
        # pair p partitions 0:64 = head 2p, 64:128 = head 2p+1
        oT_t = [
            oT_pool.tile([P, N], fmm, tag=f"oT{p}", name=f"oT{p}")
            for p in range(NPAIRS)
        ]
        nrm_pool = ctx.enter_context(tc.tile_pool(name="nrm", bufs=2))

        y_pool = ctx.enter_context(tc.tile_pool(name="y", bufs=2))

        for ic in range(N // IC):
            i0 = ic * IC
            for p in range(NPAIRS):
                qT = qkT_t[p]
                kT = qkT_t[3 + p]
                oA = psum.tile([65, IC], f32, tag="oA", name="oA")
                oB = psum.tile([65, IC], f32, tag="oB", name="oB")
                for jb in range(N // P):
                    j0 = jb * P
                    sA = psum.tile([P, IC], f32, tag="sA", name="sA")
                    sB = psum.tile([P, IC], f32, tag="sB", name="sB")
                    ptA = pt_pool.tile([P, IC], fmm, tag="ptA", name="ptA")
                    ptB = pt_pool.tile([P, IC], fmm, tag="ptB", name="ptB")
                    for u in range(IC // 512):
                        nc.tensor.matmul(
                            sA[:, u * 512:(u + 1) * 512],
                            lhsT=kT[0:64, j0:j0 + P],
                            rhs=qT[0:64, i0 + u * 512:i0 + (u + 1) * 512],
                            start=True,
                            stop=True,
                        )
                    nc.scalar.activation(ptA[:], sA[:], Exp)
                    for u in range(IC // 512):
                        nc.tensor.matmul(
                            sB[:, u * 512:(u + 1) * 512],
                            lhsT=kT[64:128, j0:j0 + P],
                            rhs=qT[64:128, i0 + u * 512:i0 + (u + 1) * 512],
                            start=True,
                            stop=True,
                        )
                    nc.scalar.activation(ptB[:], sB[:], Exp)
                    for u in range(IC // 512):
                        nc.tensor.matmul(
                            oA[:, u * 512:(u + 1) * 512],
                            lhsT=v_t[jb][:, (2 * p) * 65:(2 * p) * 65 + 65],
                            rhs=ptA[:, u * 512:(u + 1) * 512],
                            start=(jb == 0),
                            stop=(jb == N // P - 1),
                        )
                        nc.tensor.matmul(
                            oB[:, u * 512:(u + 1) * 512],
                            lhsT=v_t[jb][:, (2 * p + 1) * 65:(2 * p + 1) * 65 + 65],
                            rhs=ptB[:, u * 512:(u + 1) * 512],
                            start=(jb == 0),
                            stop=(jb == N // P - 1),
                        )
                # softmax normalization: O^T[d, i] /= rowsum[i]
                for half, o_ps in ((0, oA), (1, oB)):
                    rs = nrm_pool.tile([1, IC], f32, tag="rs", name="rs")
                    nc.vector.tensor_copy(rs[:], o_ps[64:65, :])
                    rs128 = nrm_pool.tile([P, IC // P], f32, tag="rs128",
                                          name="rs128")
                    nc.sync.dma_start(rs128[:], rs[:])
                    rcp = nrm_pool.tile([P, IC // P], f32, tag="rcp", name="rcp")
                    nc.vector.reciprocal(rcp[:], rs128[:])
                    rcpf = nrm_pool.tile([1, IC], f32, tag="rcpf", name="rcpf")
                    nc.sync.dma_start(rcpf[:], rcp[:])
                    rb = nrm_pool.tile([64, IC], f32, tag="rb", name="rb")
                    nc.gpsimd.partition_broadcast(rb[:], rcpf[:])
                    nc.vector.tensor_mul(
                        oT_t[p][half * 64:(half + 1) * 64, i0:i0 + IC],
                        o_ps[0:64, :],
                        rb[:],
                    )

            # ---- output projection for this query chunk (overlaps the next
            # chunk's attention; y psums share the oA/oB bank slots) ----
            for nb2 in range(IC // P):
                nb = ic * (IC // P) + nb2
                n0 = nb * P
                yt = y_pool.tile([P, E], f32, tag="y", name="yt")
                for half in range(2):
                    f0 = half * 384
                    psy = psum.tile(
                        [P, 384], f32, tag=("oA" if half == 0 else "oB"),
                        name="psy",
                    )
                    for dt3 in range(3):
                        nc.tensor.matmul(
                            psy[:],
                            lhsT=oT_t[dt3][:, n0:n0 + P],
                            rhs=wo_t[dt3][:, f0:f0 + 384],
                            start=(dt3 == 0),
                            stop=(dt3 == 2),
                        )
                    nc.vector.tensor_copy(yt[:, f0:f0 + 384], psy[:])
                nc.sync.dma_start(yp[n0:n0 + P, :], yt[:])

    nc.compile()
    _CACHE["nc"] = nc
    return nc


def _shard_inputs(x_q, w_qkv, b_qkv, w_out):
    """Build the 8 per-core input maps (numpy, host side)."""
    if PRECISION == "bf16":
        import ml_dtypes

        mm_np = ml_dtypes.bfloat16
    else:
        mm_np = np.float32

    def cmm(a):
        return np.ascontiguousarray(a.astype(mm_np))

    in_maps = []
    for c in range(NCORES):
        b = c // 2
        h0 = (c % 2) * HPC
        qs = slice(h0 * D, h0 * D + FQK)
        ks = slice(E + h0 * D, E + h0 * D + FQK)
        vs = slice(2 * E + h0 * D, 2 * E + h0 * D + FQK)
        wq = w_qkv[qs] * SCALE                       # (384, 768)
        wk = w_qkv[ks]
        wv = w_qkv[vs]
        in_maps.append({
            "xT": cmm(x_q[b].T),                                     # (768, 2048)
            "wqkT": cmm(np.concatenate([wq, wk], axis=0).T),         # (768, 768)
            "bq": np.ascontiguousarray(
                (b_qkv[qs] * SCALE).reshape(FQK, 1)),                # (384, 1)
            "wvT": cmm(wv.T),                                        # (768, 384)
            "woT": cmm(w_out[:, h0 * D:h0 * D + FQK].T),
        })
    return in_maps


def kernel(x_q, w_qkv, b_qkv, w_out, b_out, _trace=False, _tmpdir=None):
    x_q = np.asarray(x_q, dtype=np.float32)
    w_qkv = np.asarray(w_qkv, dtype=np.float32)
    b_qkv = np.asarray(b_qkv, dtype=np.float32)
    w_out = np.asarray(w_out, dtype=np.float32)
    b_out = np.asarray(b_out, dtype=np.float32)

    from concourse.bass_utils import run_bass_kernel_spmd

    nc = _build_bass()
    in_maps = _shard_inputs(x_q, w_qkv, b_qkv, w_out)
    res = run_bass_kernel_spmd(
        nc, in_maps, core_ids=list(range(NCORES)), trace=_trace, tmpdir=_tmpdir
    )
    _CACHE["last_result"] = res

    # host unshard: sum the two head-shards of each batch, add the folded bias
    bv = b_qkv[2 * E:]                       # v bias, folded through w_out
    b_eff = b_out + w_out @ bv               # (768,)
    y = np.empty((B, N, E), dtype=np.float32)
    for b in range(B):
        y[b] = res.results[2 * b]["yp"] + res.results[2 * b + 1]["yp"] + b_eff
    return y


# revision 12
# speedup vs baseline: 1.2571x; 1.0175x over previous
"""Multi-head attention (B=4, N=2048, E=768, H=12, D=64) on 8 TRN2 NeuronCores.

Sharding: data-parallel on batch (4 batches x 2 cores each), tensor-parallel on
heads (6 heads per core).  Each core computes its heads' full NxN attention.
Partial output projections from the two cores of a batch are summed on the host.

Math simplifications (all exact):
  - softmax is shift invariant -> drop the +1.0 score bias and max-subtraction
    (scores are ~N(0,1); exp never overflows fp32)
  - K bias adds a per-query constant to every score row -> softmax invariant -> dropped
  - V bias shifts every attention output row by bv (softmax rows sum to 1)
    -> folded into the output bias on the host: b_eff = b_out + w_out @ bv
  - q scaling (1/8) folded into Wq and bq on the host

Device layout: scores are computed transposed (S^T = K Q^T, partition = key),
so P^T = exp(S^T) feeds the P@V matmul directly as the moving operand with
V as the stationary operand (O^T = (P V)^T accumulated over key blocks).
A ones-column appended to V (M=65) yields the softmax row-sums in the same
PE stream.  Matmul operands are bf16 (fp32 PSUM accumulation) -- the PE
streams 2 bytes/cycle/partition, so 4-byte fp32r runs at half rate.  Set
PRECISION = "f32r" for full fp32r (~2x slower, ~10x lower error).
"""

import sys

if "/opt/trn_rl_repo" not in sys.path:
    sys.path.insert(0, "/opt/trn_rl_repo")

import numpy as np

B, N, E = 4, 2048, 768
H, D = 12, 64
HPC = 6                     # heads per core
FQK = HPC * D               # 384 q (or k) features per core
NCORES = 8
SCALE = D ** -0.5
PRECISION = "bf16"          # "bf16" | "f32r" (matmul operand dtype)

_CACHE = {}


def _build_bass():
    """Build the SPMD Bass program (same program on all 8 cores)."""
    if "nc" in _CACHE:
        return _CACHE["nc"]

    from contextlib import ExitStack

    import concourse.bass as bass
    import concourse.tile as tile
    from concourse import bacc, mybir

    f32 = mybir.dt.float32
    fmm = mybir.dt.bfloat16 if PRECISION == "bf16" else mybir.dt.float32r
    Identity = mybir.ActivationFunctionType.Identity
    Exp = mybir.ActivationFunctionType.Exp

    nc = bacc.Bacc(
        "TRN2",
        target_bir_lowering=False,
        debug=False,
        num_devices=NCORES,
    )

    xT = nc.dram_tensor("xT", (E, N), fmm, kind="ExternalInput").ap()        # x[b].T
    wqkT = nc.dram_tensor("wqkT", (E, 2 * FQK), fmm, kind="ExternalInput").ap()
    bq = nc.dram_tensor("bq", (FQK, 1), f32, kind="ExternalInput").ap()
    wvT = nc.dram_tensor("wvT", (E, FQK), fmm, kind="ExternalInput").ap()
    woT = nc.dram_tensor("woT", (FQK, E), fmm, kind="ExternalInput").ap()
    yp = nc.dram_tensor("yp", (N, E), f32, kind="ExternalOutput").ap()

    P = 128
    NCHUNK = 512            # token chunk for the projections
    IC = 1024               # query chunk in attention
    NPAIRS = HPC // 2       # head pairs (row-packed in the PE array)

    with ExitStack() as ctx:
        tc = ctx.enter_context(tile.TileContext(nc))

        # ---- persistent weight tiles -------------------------------------
        wpool = ctx.enter_context(tc.tile_pool(name="w", bufs=1))
        wqk_t = []
        wv_t = []
        for t in range(6):
            wqk = wpool.tile([P, 2 * FQK], fmm, tag=f"wqk{t}", name=f"wqk{t}")
            nc.sync.dma_start(wqk[:], wqkT[t * P:(t + 1) * P, :])
            wqk_t.append(wqk)
            wv = wpool.tile([P, FQK], fmm, tag=f"wv{t}", name=f"wv{t}")
            nc.sync.dma_start(wv[:], wvT[t * P:(t + 1) * P, :])
            wv_t.append(wv)
        bqt = wpool.tile([P, 3], f32, tag="bq", name="bqt")
        for fb in range(3):
            nc.sync.dma_start(bqt[:, fb:fb + 1], bq[fb * P:(fb + 1) * P, :])

        # x^T resident in full: 6 e-tiles x [128, 2048]
        xe_t = []
        for t in range(6):
            xe = wpool.tile([P, N], fmm, tag=f"xe{t}", name=f"xe{t}")
            nc.sync.dma_start(xe[:], xT[t * P:(t + 1) * P, :])
            xe_t.append(xe)

        # ---- persistent activation tiles ---------------------------------
        qk_pool = ctx.enter_context(tc.tile_pool(name="qk", bufs=1))
        # f-blocks 0..2 = q features (heads 2fb, 2fb+1), 3..5 = k features
        qkT_t = [
            qk_pool.tile([P, N], fmm, tag=f"qk{fb}", name=f"qkT{fb}")
            for fb in range(6)
        ]
        v_pool = ctx.enter_context(tc.tile_pool(name="v", bufs=1))
        # V' per key-block: [128 keys, 6*65] = per head 64 V cols + a ones col
        v_t = [
            v_pool.tile([P, HPC * 65], fmm, tag=f"v{nb}", name=f"vv{nb}")
            for nb in range(N // P)
        ]
        for nb in range(N // P):
            nc.gpsimd.memset(v_t[nb][:], 1.0)

        # output-projection weights (not needed until the first out-proj)
        wo_t = []
        for t in range(3):
            wo = wpool.tile([P, E], fmm, tag=f"wo{t}", name=f"wo{t}")
            nc.sync.dma_start(wo[:], woT[t * P:(t + 1) * P, :])
            wo_t.append(wo)

        # ---- PSUM pool ---------------------------------------------------
        # tags sA/sB: [128,1024] (2 banks each); tags oA/oB: [65,1024]-class
        # slots (2 banks each) -> 8 banks total.  Projection / out-proj psums
        # reuse these slots.
        psum = ctx.enter_context(tc.tile_pool(name="ps", bufs=1, space="PSUM"))

        pt_pool = ctx.enter_context(tc.tile_pool(name="pt", bufs=8))
        oT_pool = ctx.enter_context(tc.tile_pool(name="oT", bufs=1))
        # pair p partitions 0:64 = head 2p, 64:128 = head 2p+1
        oT_t = [
            oT_pool.tile([P, N], fmm, tag=f"oT{p}", name=f"oT{p}")
            for p in range(NPAIRS)
        ]
        nrm_pool = ctx.enter_context(tc.tile_pool(name="nrm", bufs=2))
        y_pool = ctx.enter_context(tc.tile_pool(name="y", bufs=2))

        def proj_qk(p):
            """q/k features for pair p -> qkT_t[p], qkT_t[3+p]."""
            for k, fb in enumerate((p, 3 + p)):
                for c4 in range(N // NCHUNK):
                    n0 = c4 * NCHUNK
                    ps = psum.tile(
                        [P, NCHUNK], f32,
                        tag=("sA" if (k * 4 + c4) % 2 == 0 else "sB"),
                        name="ps_qk",
                    )
                    for et in range(6):
                        nc.tensor.matmul(
                            ps[:],
                            lhsT=wqk_t[et][:, fb * P:(fb + 1) * P],
                            rhs=xe_t[et][:, n0:n0 + NCHUNK],
                            start=(et == 0),
                            stop=(et == 5),
                        )
                    dst = qkT_t[fb][:, n0:n0 + NCHUNK]
                    if fb < 3:
                        nc.vector.tensor_scalar_add(dst, ps[:], bqt[:, fb:fb + 1])
                    else:
                        nc.vector.tensor_copy(dst, ps[:])

        def proj_v():
            """V natural (all heads), into the ones-interleaved v tiles."""
            for nb in range(N // P):
                psv = psum.tile(
                    [P, FQK], f32, tag=("sA" if nb % 2 == 0 else "sB"),
                    name="ps_v",
                )
                for et in range(6):
                    nc.tensor.matmul(
                        psv[:],
                        lhsT=xe_t[et][:, nb * P:(nb + 1) * P],
                        rhs=wv_t[et][:],
                        start=(et == 0),
                        stop=(et == 5),
                    )
                v3 = v_t[nb].rearrange("p (h c) -> p h c", c=65)
                nc.vector.tensor_copy(
                    v3[:, :, 0:64],
                    psv.rearrange("p (h c) -> p h c", c=64),
                )

        def attention(p, ic):
            i0 = ic * IC
            qT = qkT_t[p]
            kT = qkT_t[3 + p]
            oA = psum.tile([65, IC], f32, tag="oA", name="oA")
            oB = psum.tile([65, IC], f32, tag="oB", name="oB")
            for jb in range(N // P):
                j0 = jb * P
                sA = psum.tile([P, IC], f32, tag="sA", name="sA")
                sB = psum.tile([P, IC], f32, tag="sB", name="sB")
                ptA = pt_pool.tile([P, IC], fmm, tag="ptA", name="ptA")
                ptB = pt_pool.tile([P, IC], fmm, tag="ptB", name="ptB")
                for u in range(IC // 512):
                    nc.tensor.matmul(
                        sA[:, u * 512:(u + 1) * 512],
                        lhsT=kT[0:64, j0:j0 + P],
                        rhs=qT[0:64, i0 + u * 512:i0 + (u + 1) * 512],
                        start=True,
                        stop=True,
                    )
                nc.scalar.activation(ptA[:], sA[:], Exp)
                for u in range(IC // 512):
                    nc.tensor.matmul(
                        sB[:, u * 512:(u + 1) * 512],
                        lhsT=kT[64:128, j0:j0 + P],
                        rhs=qT[64:128, i0 + u * 512:i0 + (u + 1) * 512],
                        start=True,
                        stop=True,
                    )
                nc.scalar.activation(ptB[:], sB[:], Exp)
                for u in range(IC // 512):
                    nc.tensor.matmul(
                        oA[:, u * 512:(u + 1) * 512],
                        lhsT=v_t[jb][:, (2 * p) * 65:(2 * p) * 65 + 65],
                        rhs=ptA[:, u * 512:(u + 1) * 512],
                        start=(jb == 0),
                        stop=(jb == N // P - 1),
                    )
                    nc.tensor.matmul(
                        oB[:, u * 512:(u + 1) * 512],
                        lhsT=v_t[jb][:, (2 * p + 1) * 65:(2 * p + 1) * 65 + 65],
                        rhs=ptB[:, u * 512:(u + 1) * 512],
                        start=(jb == 0),
                        stop=(jb == N // P - 1),
                    )
            # softmax normalization: O^T[d, i] /= rowsum[i]
            for half, o_ps in ((0, oA), (1, oB)):
                rs = nrm_pool.tile([1, IC], f32, tag="rs", name="rs")
                nc.vector.tensor_copy(rs[:], o_ps[64:65, :])
                rs128 = nrm_pool.tile([P, IC // P], f32, tag="rs128",
                                      name="rs128")
                nc.sync.dma_start(rs128[:], rs[:])
                rcp = nrm_pool.tile([P, IC // P], f32, tag="rcp", name="rcp")
                nc.vector.reciprocal(rcp[:], rs128[:])
                rcpf = nrm_pool.tile([1, IC], f32, tag="rcpf", name="rcpf")
                nc.sync.dma_start(rcpf[:], rcp[:])
                rb = nrm_pool.tile([64, IC], f32, tag="rb", name="rb")
                nc.gpsimd.partition_broadcast(rb[:], rcpf[:])
                nc.vector.tensor_mul(
                    oT_t[p][half * 64:(half + 1) * 64, i0:i0 + IC],
                    o_ps[0:64, :],
                    rb[:],
                )

        def out_proj(ic):
            """Output projection for query chunk ic (needs all pairs' oT)."""
            for nb2 in range(IC // P):
                n0 = ic * IC + nb2 * P
                yt = y_pool.tile([P, E], f32, tag="y", name="yt")
                for half in range(2):
                    f0 = half * 384
                    psy = psum.tile(
                        [P, 384], f32, tag=("oA" if half == 0 else "oB"),
                        name="psy",
                    )
                    for dt3 in range(3):
                        nc.tensor.matmul(
                            psy[:],
                            lhsT=oT_t[dt3][:, n0:n0 + P],
                            rhs=wo_t[dt3][:, f0:f0 + 384],
                            start=(dt3 == 0),
                            stop=(dt3 == 2),
                        )
                    nc.vector.tensor_copy(yt[:, f0:f0 + 384], psy[:])
                nc.sync.dma_start(yp[n0:n0 + P, :], yt[:])

        # emission order = scheduler priority: pair-p projections are emitted
        # just before pair-p attention so later-pair projections fill PE slack
        # during the ACT-bound attention stretches.
        proj_qk(0)
        proj_v()
        attention(0, 0)
        attention(0, 1)
        proj_qk(1)
        attention(1, 0)
        attention(1, 1)
        proj_qk(2)
        attention(2, 0)
        out_proj(0)
        attention(2, 1)
        out_proj(1)

    nc.compile()
    _CACHE["nc"] = nc
    return nc


def _shard_inputs(x_q, w_qkv, b_qkv, w_out):
    """Build the 8 per-core input maps (numpy, host side)."""
    if PRECISION == "bf16":
        import ml_dtypes

        mm_np = ml_dtypes.bfloat16
    else:
        mm_np = np.float32

    def cmm(a):
        return np.ascontiguousarray(a.astype(mm_np))

    in_maps = []
    for c in range(NCORES):
        b = c // 2
        h0 = (c % 2) * HPC
        qs = slice(h0 * D, h0 * D + FQK)
        ks = slice(E + h0 * D, E + h0 * D + FQK)
        vs = slice(2 * E + h0 * D, 2 * E + h0 * D + FQK)
        wq = w_qkv[qs] * SCALE                       # (384, 768)
        wk = w_qkv[ks]
        wv = w_qkv[vs]
        in_maps.append({
            "xT": cmm(x_q[b].T),                                     # (768, 2048)
            "wqkT": cmm(np.concatenate([wq, wk], axis=0).T),         # (768, 768)
            "bq": np.ascontiguousarray(
                (b_qkv[qs] * SCALE).reshape(FQK, 1)),                # (384, 1)
            "wvT": cmm(wv.T),                                        # (768, 384)
            "woT": cmm(w_out[:, h0 * D:h0 * D + FQK].T),
        })
    return in_maps


def kernel(x_q, w_qkv, b_qkv, w_out, b_out, _trace=False, _tmpdir=None):
    x_q = np.asarray(x_q, dtype=np.float32)
    w_qkv = np.asarray(w_qkv, dtype=np.float32)
    b_qkv = np.asarray(b_qkv, dtype=np.float32)
    w_out = np.asarray(w_out, dtype=np.float32)
    b_out = np.asarray(b_out, dtype=np.float32)

    from concourse.bass_utils import run_bass_kernel_spmd

    nc = _build_bass()
    in_maps = _shard_inputs(x_q, w_qkv, b_qkv, w_out)
    res = run_bass_kernel_spmd(
        nc, in_maps, core_ids=list(range(NCORES)), trace=_trace, tmpdir=_tmpdir
    )
    _CACHE["last_result"] = res

    # host unshard: sum the two head-shards of each batch, add the folded bias
    bv = b_qkv[2 * E:]                       # v bias, folded through w_out
    b_eff = b_out + w_out @ bv               # (768,)
    y = np.empty((B, N, E), dtype=np.float32)
    for b in range(B):
        y[b] = res.results[2 * b]["yp"] + res.results[2 * b + 1]["yp"] + b_eff
    return y
